# revision 2
# baseline (speedup 1.0000x reference)
"""ChannelAttention Trainium2 kernel (Bass/Tile), data-parallel over batch.

Problem shapes (hardcoded):
  x      [8, 4096, 768] fp32
  w_qkv  [2304, 768]    fp32   (nn.Linear(dim, 3*dim, bias=False))
  w_proj [768, 768]     fp32
  b_proj [768]          fp32
  out    [8, 4096, 768] fp32

Reference computation (per batch b):
  qkv = x @ w_qkv.T                      # [N, 3C]
  q, k, v: per group g (8 groups of 96 channels)
  q *= N**-0.5
  attn_g = softmax(q_g.T @ k_g, axis=-1) # [96, 96]  (contracts over N=4096)
  out_g  = attn_g @ v_g.T                # [96, N]
  y = out.T-assembled @ w_proj.T + b_proj

Sharding: batch b -> core b (8 cores, SPMD, no collectives).

Device strategy per core:
  - All matmul operands fp16 (full PE rate), accumulation fp32 in PSUM.
  - Host pre-work: cast x to fp16; transpose+cast weights (w_qkvT [c,d] with
    the q columns pre-scaled by N**-0.5; w_projT [c,e]); these are layout
    preps of the static weights plus an fp16 cast of x.
  - xT tiles ([c,n] layout, needed because qkv/proj contract over c) are
    produced by 2-byte DMA transpose (XBAR) straight from DRAM.
  - Phase 1 (per 512-token supertile): qkv matmuls; q/k -> [n,d] layout,
    v -> [d,n] layout (per-group M=96); attention logits accumulated into
    PSUM per group then added into an SBUF accumulator.
  - Phase 2: fp32 softmax over the free dim (96 logits per row), then PE
    transpose of each normalized [96,96] block.
  - Phase 3: out_gT = attn_smT.T @ vT (per group), then proj matmuls with
    per-group K=96 chunks, bias add, DMA out.
"""

import numpy as np

B, N, C = 8, 4096, 768
G = 8
GC = C // G          # 96
D3 = 3 * C           # 2304
NCORES = 8
ST = 512             # tokens per supertile
NST = N // ST        # 8
NSUB = ST // 128     # 4
CC = C // 128        # 6 contraction chunks of 128
QSCALE = float(N) ** -0.5  # 1/64, exact power of two

_CACHE = {}


def _build_nc():
    import concourse.bass as bass
    import concourse.mybir as mybir
    import concourse.tile as tile
    from concourse import bacc
    from concourse.masks import make_identity

    fp16 = mybir.dt.float16
    fp32 = mybir.dt.float32

    nc = bacc.Bacc(
        "TRN2",
        target_bir_lowering=False,
        debug=False,
        num_devices=NCORES,
    )

    xh = nc.dram_tensor("xh", [N, C], fp16, kind="ExternalInput").ap()
    wqkvT = nc.dram_tensor("wqkvT", [C, D3], fp16, kind="ExternalInput").ap()
    wprojT = nc.dram_tensor("wprojT", [C, C], fp16, kind="ExternalInput").ap()
    bproj = nc.dram_tensor("bproj", [C], fp32, kind="ExternalInput").ap()
    y = nc.dram_tensor("y", [N, C], fp32, kind="ExternalOutput").ap()

    with tile.TileContext(nc) as tc:
        from contextlib import ExitStack

        with ExitStack() as ctx:
            weights = ctx.enter_context(tc.tile_pool(name="weights", bufs=1))
            persist = ctx.enter_context(tc.tile_pool(name="persist", bufs=1))
            xt_pool = ctx.enter_context(tc.tile_pool(name="xt", bufs=12))
            qk_pool = ctx.enter_context(tc.tile_pool(name="qk", bufs=8))
            oT_pool = ctx.enter_context(tc.tile_pool(name="oT", bufs=16))
            ysb_pool = ctx.enter_context(tc.tile_pool(name="ysb", bufs=3))
            sm_pool = ctx.enter_context(tc.tile_pool(name="sm", bufs=4))
            ps_qk = ctx.enter_context(
                tc.tile_pool(name="ps_qk", bufs=4, space="PSUM")
            )
            ps_attn = ctx.enter_context(
                tc.tile_pool(name="ps_attn", bufs=2, space="PSUM")
            )
            ps_v = ctx.enter_context(
                tc.tile_pool(name="ps_v", bufs=2, space="PSUM")
            )

            # ---- static weights in SBUF ----
            wq_sb = []
            for cc_i in range(CC):
                wtile = weights.tile([128, D3], fp16, name=f"wq_{cc_i}")
                nc.scalar.dma_start(
                    out=wtile, in_=wqkvT[cc_i * 128 : (cc_i + 1) * 128, :]
                )
                wq_sb.append(wtile)
            wp_sb = []
            for g in range(G):
                wptile = weights.tile([GC, C], fp16, name=f"wp_{g}")
                nc.scalar.dma_start(
                    out=wptile, in_=wprojT[g * GC : (g + 1) * GC, :]
                )
                wp_sb.append(wptile)
            bias_sb = weights.tile([128, C], fp32, name="bias_sb")
            bias_bcast = bass.AP(
                tensor=bproj.tensor,
                offset=bproj.offset,
                ap=[[0, 128]] + [list(p) for p in bproj.ap],
            )
            nc.gpsimd.dma_start(out=bias_sb, in_=bias_bcast)
            ident = weights.tile([GC, GC], fp16, name="ident")
            make_identity(nc, ident)

            # ---- persistent intermediates ----
            vT = [persist.tile([GC, N], fp16, name=f"vT_{g}") for g in range(G)]
            attn_sb = persist.tile([GC, G * GC], fp32, name="attn_sb")
            eT = [persist.tile([GC, GC], fp16, name=f"eT_{g}") for g in range(G)]

            # ---- phase 1: qkv + attention-logit accumulation ----
            for s in range(NST):
                xts = []
                for cc_i in range(CC):
                    xt = xt_pool.tile(
                        [128, ST], fp16, tag="xt", name=f"xt_{s}_{cc_i}"
                    )
                    nc.sync.dma_start(
                        out=xt,
                        in_=xh[
                            s * ST : (s + 1) * ST, cc_i * 128 : (cc_i + 1) * 128
                        ],
                        transpose=True,
                    )
                    xts.append(xt)

                qs, ks = [], []
                for t in range(NSUB):
                    q_sb = qk_pool.tile([128, C], fp16, tag="q", name=f"q_{s}_{t}")
                    k_sb = qk_pool.tile([128, C], fp16, tag="k", name=f"k_{s}_{t}")
                    for half in range(2):
                        hsl = slice(half * 384, (half + 1) * 384)
                        q_ps = ps_qk.tile(
                            [128, 384], fp32, tag="qkps", name=f"qps_{s}_{t}_{half}"
                        )
                        k_ps = ps_qk.tile(
                            [128, 384], fp32, tag="qkps", name=f"kps_{s}_{t}_{half}"
                        )
                        for cc_i in range(CC):
                            lhsT = xts[cc_i][:, t * 128 : (t + 1) * 128]
                            nc.tensor.matmul(
                                q_ps,
                                lhsT,
                                wq_sb[cc_i][:, half * 384 : (half + 1) * 384],
                                start=(cc_i == 0),
                                stop=(cc_i == CC - 1),
                            )
                            nc.tensor.matmul(
                                k_ps,
                                lhsT,
                                wq_sb[cc_i][
                                    :, 768 + half * 384 : 768 + (half + 1) * 384
                                ],
                                start=(cc_i == 0),
                                stop=(cc_i == CC - 1),
                            )
                        nc.vector.tensor_copy(q_sb[:, hsl], q_ps)
                        nc.vector.tensor_copy(k_sb[:, hsl], k_ps)
                    qs.append(q_sb)
                    ks.append(k_sb)

                # attention logits: attn_g += q_g.T @ k_g (contract over n)
                for g in range(G):
                    gsl = slice(g * GC, (g + 1) * GC)
                    a_ps = ps_attn.tile(
                        [GC, GC], fp32, tag="attnps", name=f"aps_{s}_{g}"
                    )
                    for t in range(NSUB):
                        nc.tensor.matmul(
                            a_ps,
                            qs[t][:, gsl],
                            ks[t][:, gsl],
                            start=(t == 0),
                            stop=(t == NSUB - 1),
                        )
                    if s == 0:
                        nc.vector.tensor_copy(attn_sb[:, gsl], a_ps)
                    else:
                        nc.vector.tensor_add(
                            attn_sb[:, gsl], attn_sb[:, gsl], a_ps
                        )

                # v in [d, n] layout, per group (M=96)
                for g in range(G):
                    v_ps = ps_v.tile([GC, ST], fp32, tag="vps", name=f"vps_{s}_{g}")
                    for cc_i in range(CC):
                        nc.tensor.matmul(
                            v_ps,
                            wq_sb[cc_i][
                                :, 1536 + g * GC : 1536 + (g + 1) * GC
                            ],
                            xts[cc_i],
                            start=(cc_i == 0),
                            stop=(cc_i == CC - 1),
                        )
                    nc.vector.tensor_copy(vT[g][:, s * ST : (s + 1) * ST], v_ps)

            # ---- phase 2: softmax + transpose ----
            for g in range(G):
                gsl = slice(g * GC, (g + 1) * GC)
                nm = sm_pool.tile([GC, 1], fp32, tag="nm", name=f"nm_{g}")
                nc.vector.tensor_reduce(
                    out=nm,
                    in_=attn_sb[:, gsl],
                    axis=mybir.AxisListType.X,
                    op=mybir.AluOpType.max,
                    negate=True,
                )
                e_t = sm_pool.tile([GC, GC], fp32, tag="e", name=f"e_{g}")
                ssum = sm_pool.tile([GC, 1], fp32, tag="ssum", name=f"ssum_{g}")
                nc.scalar.activation(
                    e_t,
                    attn_sb[:, gsl],
                    mybir.ActivationFunctionType.Exp,
                    bias=nm,
                    scale=1.0,
                    accum_out=ssum,
                )
                rs = sm_pool.tile([GC, 1], fp32, tag="rs", name=f"rs_{g}")
                nc.vector.reciprocal(rs, ssum)
                e16 = sm_pool.tile([GC, GC], fp16, tag="e16", name=f"e16_{g}")
                nc.vector.tensor_scalar_mul(e16, e_t, rs)
                et_ps = ps_attn.tile(
                    [GC, GC], fp16, tag="attnps", name=f"etps_{g}"
                )
                nc.tensor.transpose(et_ps, e16, ident)
                nc.vector.tensor_copy(eT[g], et_ps)

            # ---- phase 3: out + proj + bias ----
            for s in range(NST):
                oTs = []
                for g in range(G):
                    o_ps = ps_v.tile([GC, ST], fp32, tag="vps", name=f"ops_{s}_{g}")
                    nc.tensor.matmul(
                        o_ps,
                        eT[g],
                        vT[g][:, s * ST : (s + 1) * ST],
                        start=True,
                        stop=True,
                    )
                    oT_t = oT_pool.tile(
                        [GC, ST], fp16, tag="oT", name=f"oT_{s}_{g}"
                    )
                    nc.vector.tensor_copy(oT_t, o_ps)
                    oTs.append(oT_t)
                for t in range(NSUB):
                    y_sb = ysb_pool.tile(
                        [128, C], fp32, tag="ysb", name=f"ysb_{s}_{t}"
                    )
                    for half in range(2):
                        hsl = slice(half * 384, (half + 1) * 384)
                        y_ps = ps_qk.tile(
                            [128, 384], fp32, tag="qkps", name=f"yps_{s}_{t}_{half}"
                        )
                        for g in range(G):
                            nc.tensor.matmul(
                                y_ps,
                                oTs[g][:, t * 128 : (t + 1) * 128],
                                wp_sb[g][:, hsl],
                                start=(g == 0),
                                stop=(g == G - 1),
                            )
                        nc.vector.tensor_add(
                            y_sb[:, hsl], y_ps, bias_sb[:, hsl]
                        )
                    row0 = s * ST + t * 128
                    nc.scalar.dma_start(
                        out=y[row0 : row0 + 128, :], in_=y_sb
                    )

    nc.compile()
    return nc


def _get_nc():
    if "nc" not in _CACHE:
        _CACHE["nc"] = _build_nc()
    return _CACHE["nc"]


def _host_prep(x, w_qkv, w_proj, b_proj):
    x = np.asarray(x, dtype=np.float32)
    w_qkv = np.asarray(w_qkv, dtype=np.float32)
    w_proj = np.asarray(w_proj, dtype=np.float32)
    b_proj = np.asarray(b_proj, dtype=np.float32)

    wq = w_qkv.copy()
    wq[:C, :] *= np.float32(QSCALE)  # fold q-scale into q rows (exact: 2^-6)
    wqkvT_h = np.ascontiguousarray(wq.T).astype(np.float16)       # [768, 2304]
    wprojT_h = np.ascontiguousarray(w_proj.T).astype(np.float16)  # [768, 768]

    in_maps = []
    for b in range(NCORES):
        in_maps.append(
            {
                "xh": np.ascontiguousarray(x[b]).astype(np.float16),
                "wqkvT": wqkvT_h,
                "wprojT": wprojT_h,
                "bproj": b_proj,
            }
        )
    return in_maps


def _run(in_maps, trace=False):
    from concourse.bass_utils import run_bass_kernel_spmd

    nc = _get_nc()
    res = run_bass_kernel_spmd(
        nc, in_maps, list(range(NCORES)), trace=trace
    )
    out = np.stack([res.results[i]["y"] for i in range(NCORES)], axis=0)
    return out.astype(np.float32, copy=False), res


def kernel(x, w_qkv, w_proj, b_proj):
    in_maps = _host_prep(x, w_qkv, w_proj, b_proj)
    out, _ = _run(in_maps, trace=False)
    return out


def run_profiled(x, w_qkv, w_proj, b_proj):
    """Returns (out, BassKernelResults) with NTFF profiling enabled."""
    in_maps = _host_prep(x, w_qkv, w_proj, b_proj)
    return _run(in_maps, trace=True)


# revision 3
# speedup vs baseline: 1.1480x; 1.1480x over previous
"""ChannelAttention Trainium2 kernel (Bass/Tile), data-parallel over batch.

Problem shapes (hardcoded):
  x      [8, 4096, 768] fp32
  w_qkv  [2304, 768]    fp32
  w_proj [768, 768]     fp32
  b_proj [768]          fp32
  out    [8, 4096, 768] fp32

Reference (per batch b, 8 groups of 96 channels):
  qkv = x @ w_qkv.T ; q *= N**-0.5
  attn_g = softmax(q_g.T @ k_g, axis=-1)     # [96, 96], contracts over N
  out_g  = attn_g @ v_g.T                    # [96, N]
  y = out @ w_proj.T + b_proj

Sharding: batch b -> core b (8 cores SPMD, no collectives).

Key structural ideas (v3):
  - attn logits via the Gram matrix: q.T@k = Wq_s @ (X.T X) @ Wk.T.
    G = X.T X costs 4096*768*768 MACs (with symmetry: only upper chunk
    blocks + PE-transpose mirror) instead of materializing q,k
    (4096*768*1536). x is consumed in natural [n,c] layout for G.
  - All matmul operands fp16 (full PE rate), fp32 PSUM accumulation,
    softmax in fp32.
  - xT ([c,n] layout for the v projection) via 2-byte XBAR DMA transpose
    straight from DRAM.
  - v and proj matmuls run at full array width (M=128 / K=128-aligned
    chunks of the 768 channel dim). The per-group softmax matrices are
    assembled into 128-aligned block-diagonal chunks E16[(dd,cc)] =
    attn_sm^T[128dd:+128, 128cc:+128] using partition-shifting
    SBUF->SBUF DMA copies of the PE-transposed [96,96] blocks.
  - out_T[c-chunk, n] = sum_dd E16[(dd,cc)].T @ vT6[dd]; proj contracts
    c in 6 chunks of 128.
Host pre-work: fp16 casts, weight transposes, fold N**-0.5 into Wq.
"""

import numpy as np

B, N, C = 8, 4096, 768
G = 8
GC = C // G          # 96
NCORES = 8
ST = 512             # tokens per supertile
NST = N // ST        # 8
NSUB = ST // 128     # 4
CC = C // 128        # 6 chunks of the channel dim
D3 = 3 * C
QSCALE = float(N) ** -0.5  # 1/64

# block-diagonal chunk pairs (dd, cc) where some group's [96,96] block
# intersects d-chunk dd and c-chunk cc
def _chunk_pairs():
    pairs = {}
    for g in range(G):
        lo, hi = g * GC, (g + 1) * GC
        ccs = range(lo // 128, (hi - 1) // 128 + 1)
        dds = range(lo // 128, (hi - 1) // 128 + 1)
        for cc_i in ccs:
            for dd in dds:
                pairs.setdefault((dd, cc_i), []).append(g)
    return pairs


CHUNK_PAIRS = _chunk_pairs()  # {(dd, cc): [groups]}

_CACHE = {}


def _build_nc():
    import concourse.bass as bass
    import concourse.mybir as mybir
    import concourse.tile as tile
    from concourse import bacc
    from concourse.masks import make_identity

    fp16 = mybir.dt.float16
    fp32 = mybir.dt.float32

    nc = bacc.Bacc(
        "TRN2", target_bir_lowering=False, debug=False, num_devices=NCORES
    )

    xh = nc.dram_tensor("xh", [N, C], fp16, kind="ExternalInput").ap()
    wqkvT = nc.dram_tensor("wqkvT", [C, D3], fp16, kind="ExternalInput").ap()
    wprojT = nc.dram_tensor("wprojT", [C, C], fp16, kind="ExternalInput").ap()
    bproj = nc.dram_tensor("bproj", [C], fp32, kind="ExternalInput").ap()
    y = nc.dram_tensor("y", [N, C], fp32, kind="ExternalOutput").ap()

    with tile.TileContext(nc) as tc:
        from contextlib import ExitStack

        with ExitStack() as ctx:
            weights = ctx.enter_context(tc.tile_pool(name="weights", bufs=1))
            persist = ctx.enter_context(tc.tile_pool(name="persist", bufs=1))
            xn_pool = ctx.enter_context(tc.tile_pool(name="xn", bufs=6))
            xt_pool = ctx.enter_context(tc.tile_pool(name="xt", bufs=12))
            oT_pool = ctx.enter_context(tc.tile_pool(name="oT", bufs=12))
            ysb_pool = ctx.enter_context(tc.tile_pool(name="ysb", bufs=3))
            sm_pool = ctx.enter_context(tc.tile_pool(name="sm", bufs=4))
            m1_pool = ctx.enter_context(tc.tile_pool(name="m1", bufs=1))
            ps_gram = ctx.enter_context(
                tc.tile_pool(name="ps_gram", bufs=3, space="PSUM")
            )
            ps_v = ctx.enter_context(
                tc.tile_pool(name="ps_v", bufs=3, space="PSUM")
            )
            ps_sm = ctx.enter_context(
                tc.tile_pool(name="ps_sm", bufs=2, space="PSUM")
            )

            # ---- static weights (SWDGE so they don't xbar-serialize with
            # the transpose DMAs) ----
            wq_sb = []
            for cc_i in range(CC):
                wtile = weights.tile([128, D3], fp16, name=f"wq_{cc_i}")
                nc.gpsimd.dma_start(
                    out=wtile, in_=wqkvT[cc_i * 128 : (cc_i + 1) * 128, :]
                )
                wq_sb.append(wtile)
            wp_sb = []
            for cc_i in range(CC):
                wptile = weights.tile([128, C], fp16, name=f"wp_{cc_i}")
                nc.gpsimd.dma_start(
                    out=wptile, in_=wprojT[cc_i * 128 : (cc_i + 1) * 128, :]
                )
                wp_sb.append(wptile)
            bias_sb = weights.tile([128, C], fp32, name="bias_sb")
            bias_bcast = bass.AP(
                tensor=bproj.tensor,
                offset=bproj.offset,
                ap=[[0, 128]] + [list(p) for p in bproj.ap],
            )
            nc.gpsimd.dma_start(out=bias_sb, in_=bias_bcast)
            ident16 = weights.tile([GC, GC], fp16, name="ident16")
            make_identity(nc, ident16)
            ident32 = weights.tile([128, 128], fp32, name="ident32")
            make_identity(nc, ident32)

            # ---- persistent intermediates ----
            # G rows chunk a: cols a*128..768 computed, lower mirrored later
            G_sb = [
                persist.tile([128, C], fp32, name=f"G_{a}") for a in range(CC)
            ]
            G16 = [
                persist.tile([128, C], fp16, name=f"G16_{a}") for a in range(CC)
            ]
            vT6 = [
                persist.tile([128, N], fp16, name=f"vT_{dd}") for dd in range(CC)
            ]
            eT = [persist.tile([GC, GC], fp16, name=f"eT_{g}") for g in range(G)]
            E16 = {}
            for (dd, cc_i) in CHUNK_PAIRS:
                t_ = persist.tile([128, 128], fp16, name=f"E_{dd}_{cc_i}")
                nc.vector.memset(t_, 0.0)
                E16[(dd, cc_i)] = t_

            # gram upper-block N-slices: for lhsT chunk a, rhs cols
            # a*128..768 split into pieces of <=384
            def gram_slices(a):
                out = []
                off = a * 128
                while off < C:
                    w = min(384, C - off)
                    out.append((off, w))
                    off += w
                return out

            # ---- phase 1: Gram accumulation + v projection ----
            for s in range(NST):
                xn = []
                for t in range(NSUB):
                    xtile = xn_pool.tile(
                        [128, C], fp16, tag="xn", name=f"xn_{s}_{t}"
                    )
                    r0 = s * ST + t * 128
                    nc.scalar.dma_start(out=xtile, in_=xh[r0 : r0 + 128, :])
                    xn.append(xtile)
                xts = []
                for cc_i in range(CC):
                    xt = xt_pool.tile(
                        [128, ST], fp16, tag="xt", name=f"xt_{s}_{cc_i}"
                    )
                    nc.sync.dma_start(
                        out=xt,
                        in_=xh[
                            s * ST : (s + 1) * ST, cc_i * 128 : (cc_i + 1) * 128
                        ],
                        transpose=True,
                    )
                    xts.append(xt)

                # gram: G[a, off:off+w] += sum_t xn[t][:,a-chunk].T @ xn[t][:,off:off+w]
                for a in range(CC):
                    for (off, w) in gram_slices(a):
                        g_ps = ps_gram.tile(
                            [128, 384], fp32, tag="gram", name=f"gps_{s}_{a}_{off}"
                        )
                        for t in range(NSUB):
                            nc.tensor.matmul(
                                g_ps[:, :w],
                                xn[t][:, a * 128 : (a + 1) * 128],
                                xn[t][:, off : off + w],
                                start=(t == 0),
                                stop=(t == NSUB - 1),
                            )
                        if s == 0:
                            nc.vector.tensor_copy(
                                G_sb[a][:, off : off + w], g_ps[:, :w]
                            )
                        else:
                            nc.vector.tensor_add(
                                G_sb[a][:, off : off + w],
                                G_sb[a][:, off : off + w],
                                g_ps[:, :w],
                            )

                # v in [d, n] layout, full-width d-chunks (v cols of wqkvT
                # are 128-aligned: 1536 + dd*128)
                for dd in range(CC):
                    v_ps = ps_v.tile(
                        [128, ST], fp32, tag="vps", name=f"vps_{s}_{dd}"
                    )
                    for cc_i in range(CC):
                        nc.tensor.matmul(
                            v_ps,
                            wq_sb[cc_i][:, 1536 + dd * 128 : 1536 + (dd + 1) * 128],
                            xts[cc_i],
                            start=(cc_i == 0),
                            stop=(cc_i == CC - 1),
                        )
                    nc.vector.tensor_copy(vT6[dd][:, s * ST : (s + 1) * ST], v_ps)

            # ---- phase 2a: mirror G lower blocks + cast to fp16 ----
            for a in range(CC):
                for b_ in range(a + 1, CC):
                    # G(b_, a) = G(a, b_)^T
                    m_ps = ps_gram.tile(
                        [128, 128], fp32, tag="gram", name=f"mir_{a}_{b_}"
                    )
                    nc.tensor.transpose(
                        m_ps, G_sb[a][:, b_ * 128 : (b_ + 1) * 128], ident32
                    )
                    nc.vector.tensor_copy(
                        G_sb[b_][:, a * 128 : (a + 1) * 128], m_ps
                    )
            for a in range(CC):
                nc.vector.tensor_copy(G16[a], G_sb[a])

            # ---- phase 2b: M1_g = G @ Wk_g ; A_g = Wqs_g^T @ M1_g ----
            for g in range(G):
                m1_g = []
                for a in range(CC):
                    m1_ps = ps_v.tile(
                        [128, GC], fp32, tag="vps", name=f"m1ps_{g}_{a}"
                    )
                    for b_ in range(CC):
                        nc.tensor.matmul(
                            m1_ps,
                            G16[b_][:, a * 128 : (a + 1) * 128],
                            wq_sb[b_][:, 768 + g * GC : 768 + (g + 1) * GC],
                            start=(b_ == 0),
                            stop=(b_ == CC - 1),
                        )
                    m1_t = m1_pool.tile(
                        [128, GC], fp16, tag="m1", name=f"m1_{g}_{a}"
                    )
                    nc.vector.tensor_copy(m1_t, m1_ps)
                    m1_g.append(m1_t)

                a_ps = ps_sm.tile([GC, GC], fp32, tag="aps", name=f"aps_{g}")
                for a in range(CC):
                    nc.tensor.matmul(
                        a_ps,
                        wq_sb[a][:, g * GC : (g + 1) * GC],
                        m1_g[a],
                        start=(a == 0),
                        stop=(a == CC - 1),
                    )

                # softmax over free dim + transpose
                nm = sm_pool.tile([GC, 1], fp32, tag="nm", name=f"nm_{g}")
                nc.vector.tensor_reduce(
                    out=nm,
                    in_=a_ps,
                    axis=mybir.AxisListType.X,
                    op=mybir.AluOpType.max,
                    negate=True,
                )
                e_t = sm_pool.tile([GC, GC], fp32, tag="e", name=f"e_{g}")
                ssum = sm_pool.tile([GC, 1], fp32, tag="ssum", name=f"ssum_{g}")
                nc.scalar.activation(
                    e_t,
                    a_ps,
                    mybir.ActivationFunctionType.Exp,
                    bias=nm,
                    scale=1.0,
                    accum_out=ssum,
                )
                rs = sm_pool.tile([GC, 1], fp32, tag="rs", name=f"rs_{g}")
                nc.vector.reciprocal(rs, ssum)
                e16 = sm_pool.tile([GC, GC], fp16, tag="e16", name=f"e16_{g}")
                nc.vector.tensor_scalar_mul(e16, e_t, rs)
                et_ps = ps_sm.tile([GC, GC], fp16, tag="aps", name=f"etps_{g}")
                nc.tensor.transpose(et_ps, e16, ident16)
                nc.vector.tensor_copy(eT[g], et_ps)

            # ---- phase 2c: scatter eT blocks into block-diagonal E16 ----
            # eT[g][r, q] = attn_sm[96g+q, 96g+r]
            # E16[(dd,cc)][p, m] = attn_sm[128cc+m, 128dd+p] for matching g
            for g in range(G):
                lo, hi = g * GC, (g + 1) * GC
                for dd in range(lo // 128, (hi - 1) // 128 + 1):
                    r0 = max(0, 128 * dd - lo)
                    r1 = min(GC, 128 * (dd + 1) - lo)
                    for cc_i in range(lo // 128, (hi - 1) // 128 + 1):
                        q0 = max(0, 128 * cc_i - lo)
                        q1 = min(GC, 128 * (cc_i + 1) - lo)
                        nc.sync.dma_start(
                            out=E16[(dd, cc_i)][
                                lo + r0 - 128 * dd : lo + r1 - 128 * dd,
                                lo + q0 - 128 * cc_i : lo + q1 - 128 * cc_i,
                            ],
                            in_=eT[g][r0:r1, q0:q1],
                        )

            # ---- phase 3: out_T = E^T @ vT (per c-chunk) ; proj ; bias ----
            for s in range(NST):
                oT6 = []
                for cc_i in range(CC):
                    dds = sorted(dd for (dd, c2) in CHUNK_PAIRS if c2 == cc_i)
                    o_ps = ps_v.tile(
                        [128, ST], fp32, tag="vps", name=f"ops_{s}_{cc_i}"
                    )
                    for j, dd in enumerate(dds):
                        nc.tensor.matmul(
                            o_ps,
                            E16[(dd, cc_i)],
                            vT6[dd][:, s * ST : (s + 1) * ST],
                            start=(j == 0),
                            stop=(j == len(dds) - 1),
                        )
                    oT_t = oT_pool.tile(
                        [128, ST], fp16, tag="oT", name=f"oT_{s}_{cc_i}"
                    )
                    nc.vector.tensor_copy(oT_t, o_ps)
                    oT6.append(oT_t)
                for t in range(NSUB):
                    y_sb = ysb_pool.tile(
                        [128, C], fp32, tag="ysb", name=f"ysb_{s}_{t}"
                    )
                    for half in range(2):
                        hsl = slice(half * 384, (half + 1) * 384)
                        y_ps = ps_gram.tile(
                            [128, 384], fp32, tag="gram", name=f"yps_{s}_{t}_{half}"
                        )
                        for cc_i in range(CC):
                            nc.tensor.matmul(
                                y_ps,
                                oT6[cc_i][:, t * 128 : (t + 1) * 128],
                                wp_sb[cc_i][:, hsl],
                                start=(cc_i == 0),
                                stop=(cc_i == CC - 1),
                            )
                        nc.vector.tensor_add(y_sb[:, hsl], y_ps, bias_sb[:, hsl])
                    row0 = s * ST + t * 128
                    nc.scalar.dma_start(out=y[row0 : row0 + 128, :], in_=y_sb)

    nc.compile()
    return nc


def _get_nc():
    if "nc" not in _CACHE:
        _CACHE["nc"] = _build_nc()
    return _CACHE["nc"]


def _host_prep(x, w_qkv, w_proj, b_proj):
    x = np.asarray(x, dtype=np.float32)
    w_qkv = np.asarray(w_qkv, dtype=np.float32)
    w_proj = np.asarray(w_proj, dtype=np.float32)
    b_proj = np.asarray(b_proj, dtype=np.float32)

    wq = w_qkv.copy()
    wq[:C, :] *= np.float32(QSCALE)
    wqkvT_h = np.ascontiguousarray(wq.T).astype(np.float16)       # [768, 2304]
    wprojT_h = np.ascontiguousarray(w_proj.T).astype(np.float16)  # [768, 768]

    in_maps = []
    for b_ in range(NCORES):
        in_maps.append(
            {
                "xh": np.ascontiguousarray(x[b_]).astype(np.float16),
                "wqkvT": wqkvT_h,
                "wprojT": wprojT_h,
                "bproj": b_proj,
            }
        )
    return in_maps


def _run(in_maps, trace=False):
    from concourse.bass_utils import run_bass_kernel_spmd

    nc = _get_nc()
    res = run_bass_kernel_spmd(nc, in_maps, list(range(NCORES)), trace=trace)
    out = np.stack([res.results[i]["y"] for i in range(NCORES)], axis=0)
    return out.astype(np.float32, copy=False), res


def kernel(x, w_qkv, w_proj, b_proj):
    in_maps = _host_prep(x, w_qkv, w_proj, b_proj)
    out, _ = _run(in_maps, trace=False)
    return out


def run_profiled(x, w_qkv, w_proj, b_proj):
    """Returns (out, BassKernelResults) with NTFF profiling enabled."""
    in_maps = _host_prep(x, w_qkv, w_proj, b_proj)
    return _run(in_maps, trace=True)


# revision 9
# speedup vs baseline: 1.2921x; 1.1256x over previous
"""ChannelAttention Trainium2 kernel (Bass/Tile), data-parallel over batch.

Problem shapes (hardcoded):
  x      [8, 4096, 768] fp32
  w_qkv  [2304, 768]    fp32
  w_proj [768, 768]     fp32
  b_proj [768]          fp32
  out    [8, 4096, 768] fp32

Reference (per batch b, 8 groups of 96 channels):
  qkv = x @ w_qkv.T ; q *= N**-0.5
  attn_g = softmax(q_g.T @ k_g, axis=-1)     # [96, 96], contracts over N
  out_g  = attn_g @ v_g.T                    # [96, N]
  y = out @ w_proj.T + b_proj

Sharding: batch b -> core b (8 cores SPMD, no collectives).

Key structural ideas (v3):
  - attn logits via the Gram matrix: q.T@k = Wq_s @ (X.T X) @ Wk.T.
    G = X.T X costs 4096*768*768 MACs (with symmetry: only upper chunk
    blocks + PE-transpose mirror) instead of materializing q,k
    (4096*768*1536). x is consumed in natural [n,c] layout for G.
  - All matmul operands fp16 (full PE rate), fp32 PSUM accumulation,
    softmax in fp32.
  - xT ([c,n] layout for the v projection) via 2-byte XBAR DMA transpose
    straight from DRAM.
  - v and proj matmuls run at full array width (M=128 / K=128-aligned
    chunks of the 768 channel dim). The per-group softmax matrices are
    assembled into 128-aligned block-diagonal chunks E16[(dd,cc)] =
    attn_sm^T[128dd:+128, 128cc:+128] using partition-shifting
    SBUF->SBUF DMA copies of the PE-transposed [96,96] blocks.
  - out_T[c-chunk, n] = sum_dd E16[(dd,cc)].T @ vT6[dd]; proj contracts
    c in 6 chunks of 128.
Host pre-work: fp16 casts, weight transposes, fold N**-0.5 into Wq.
"""

import numpy as np

B, N, C = 8, 4096, 768
G = 8
GC = C // G          # 96
NCORES = 8
ST = 512             # tokens per supertile
NST = N // ST        # 8
NSUB = ST // 128     # 4
CC = C // 128        # 6 chunks of the channel dim
D3 = 3 * C
QSCALE = float(N) ** -0.5  # 1/64

# block-diagonal chunk pairs (dd, cc) where some group's [96,96] block
# intersects d-chunk dd and c-chunk cc
def _chunk_pairs():
    pairs = {}
    for g in range(G):
        lo, hi = g * GC, (g + 1) * GC
        ccs = range(lo // 128, (hi - 1) // 128 + 1)
        dds = range(lo // 128, (hi - 1) // 128 + 1)
        for cc_i in ccs:
            for dd in dds:
                pairs.setdefault((dd, cc_i), []).append(g)
    return pairs


CHUNK_PAIRS = _chunk_pairs()  # {(dd, cc): [groups]}

_CACHE = {}


def _build_nc():
    import concourse.bass as bass
    import concourse.mybir as mybir
    import concourse.tile as tile
    from concourse import bacc
    from concourse.masks import make_identity

    fp16 = mybir.dt.float16
    fp32 = mybir.dt.float32

    nc = bacc.Bacc(
        "TRN2", target_bir_lowering=False, debug=False, num_devices=NCORES
    )

    xh = nc.dram_tensor("xh", [N, C], fp16, kind="ExternalInput").ap()
    wqkvT = nc.dram_tensor("wqkvT", [C, D3], fp16, kind="ExternalInput").ap()
    wprojT = nc.dram_tensor("wprojT", [C, C], fp16, kind="ExternalInput").ap()
    bproj = nc.dram_tensor("bproj", [C], fp32, kind="ExternalInput").ap()
    y = nc.dram_tensor("y", [N, C], fp32, kind="ExternalOutput").ap()

    with tile.TileContext(nc) as tc:
        from contextlib import ExitStack

        with ExitStack() as ctx:
            weights = ctx.enter_context(tc.tile_pool(name="weights", bufs=1))
            persist = ctx.enter_context(tc.tile_pool(name="persist", bufs=1))
            xn_pool = ctx.enter_context(tc.tile_pool(name="xn", bufs=6))
            xt_pool = ctx.enter_context(tc.tile_pool(name="xt", bufs=12))
            oT_pool = ctx.enter_context(tc.tile_pool(name="oT", bufs=12))
            ysb_pool = ctx.enter_context(tc.tile_pool(name="ysb", bufs=3))
            sm_pool = ctx.enter_context(tc.tile_pool(name="sm", bufs=4))
            m1_pool = ctx.enter_context(tc.tile_pool(name="m1", bufs=1))
            ps_gram = ctx.enter_context(
                tc.tile_pool(name="ps_gram", bufs=3, space="PSUM")
            )
            ps_v = ctx.enter_context(
                tc.tile_pool(name="ps_v", bufs=3, space="PSUM")
            )
            ps_tp = ctx.enter_context(
                tc.tile_pool(name="ps_tp", bufs=2, space="PSUM")
            )

            # ---- static weights ----
            wq_sb = []
            for cc_i in range(CC):
                wtile = weights.tile([128, D3], fp16, name=f"wq_{cc_i}")
                nc.sync.dma_start(
                    out=wtile, in_=wqkvT[cc_i * 128 : (cc_i + 1) * 128, :]
                )
                wq_sb.append(wtile)
            wp_sb = []
            for cc_i in range(CC):
                wptile = weights.tile([128, C], fp16, name=f"wp_{cc_i}")
                nc.sync.dma_start(
                    out=wptile, in_=wprojT[cc_i * 128 : (cc_i + 1) * 128, :]
                )
                wp_sb.append(wptile)
            bias_sb = weights.tile([128, C], fp32, name="bias_sb")
            bias_bcast = bass.AP(
                tensor=bproj.tensor,
                offset=bproj.offset,
                ap=[[0, 128]] + [list(p) for p in bproj.ap],
            )
            nc.gpsimd.dma_start(out=bias_sb, in_=bias_bcast)
            ident16 = weights.tile([128, 128], fp16, name="ident16")
            make_identity(nc, ident16)
            ident32 = weights.tile([128, 128], fp32, name="ident32")
            make_identity(nc, ident32)

            # ---- persistent intermediates ----
            # G rows chunk a: cols a*128..768 computed, lower mirrored later
            G_sb = [
                persist.tile([128, C], fp32, name=f"G_{a}") for a in range(CC)
            ]
            G16 = [
                persist.tile([128, C], fp16, name=f"G16_{a}") for a in range(CC)
            ]
            vT6 = [
                persist.tile([128, N], fp16, name=f"vT_{dd}") for dd in range(CC)
            ]
            eT = [persist.tile([GC, GC], fp16, name=f"eT_{g}") for g in range(G)]
            E16 = {}
            for (dd, cc_i) in CHUNK_PAIRS:
                t_ = persist.tile([128, 128], fp16, name=f"E_{dd}_{cc_i}")
                nc.vector.memset(t_, 0.0)
                E16[(dd, cc_i)] = t_

            # gram upper-block N-slices: for lhsT chunk a, rhs cols
            # a*128..768 split into pieces of <=384
            def gram_slices(a):
                out = []
                off = a * 128
                while off < C:
                    w = min(384, C - off)
                    out.append((off, w))
                    off += w
                return out

            # ---- phase 1: Gram accumulation + v projection ----
            for s in range(NST):
                xn = []
                for t in range(NSUB):
                    xtile = xn_pool.tile(
                        [128, C], fp16, tag="xn", name=f"xn_{s}_{t}"
                    )
                    r0 = s * ST + t * 128
                    nc.scalar.dma_start(out=xtile, in_=xh[r0 : r0 + 128, :])
                    xn.append(xtile)
                # xT via PE transpose of the natural tiles
                xts = [
                    xt_pool.tile([128, ST], fp16, tag="xt", name=f"xt_{s}_{cc_i}")
                    for cc_i in range(CC)
                ]
                for t in range(NSUB):
                    for cc_i in range(CC):
                        tp_ps = ps_tp.tile(
                            [128, 128], fp16, tag="tp", name=f"tp_{s}_{t}_{cc_i}"
                        )
                        nc.tensor.transpose(
                            tp_ps,
                            xn[t][:, cc_i * 128 : (cc_i + 1) * 128],
                            ident16,
                        )
                        nc.vector.tensor_copy(
                            xts[cc_i][:, t * 128 : (t + 1) * 128], tp_ps
                        )

                # gram: G[a, off:off+w] += sum_t xn[t][:,a-chunk].T @ xn[t][:,off:off+w]
                for a in range(CC):
                    for (off, w) in gram_slices(a):
                        g_ps = ps_gram.tile(
                            [128, 384], fp32, tag="gram", name=f"gps_{s}_{a}_{off}"
                        )
                        for t in range(NSUB):
                            nc.tensor.matmul(
                                g_ps[:, :w],
                                xn[t][:, a * 128 : (a + 1) * 128],
                                xn[t][:, off : off + w],
                                start=(t == 0),
                                stop=(t == NSUB - 1),
                            )
                        if s == 0:
                            nc.vector.tensor_copy(
                                G_sb[a][:, off : off + w], g_ps[:, :w]
                            )
                        else:
                            nc.vector.tensor_add(
                                G_sb[a][:, off : off + w],
                                G_sb[a][:, off : off + w],
                                g_ps[:, :w],
                            )

                # v in [d, n] layout, full-width d-chunks (v cols of wqkvT
                # are 128-aligned: 1536 + dd*128)
                for dd in range(CC):
                    v_ps = ps_v.tile(
                        [128, ST], fp32, tag="vps", name=f"vps_{s}_{dd}"
                    )
                    for cc_i in range(CC):
                        nc.tensor.matmul(
                            v_ps,
                            wq_sb[cc_i][:, 1536 + dd * 128 : 1536 + (dd + 1) * 128],
                            xts[cc_i],
                            start=(cc_i == 0),
                            stop=(cc_i == CC - 1),
                        )
                    nc.vector.tensor_copy(vT6[dd][:, s * ST : (s + 1) * ST], v_ps)

            # ---- phase 2a: mirror G lower blocks + cast to fp16 ----
            for a in range(CC):
                for b_ in range(a + 1, CC):
                    # G(b_, a) = G(a, b_)^T
                    m_ps = ps_gram.tile(
                        [128, 128], fp32, tag="gram", name=f"mir_{a}_{b_}"
                    )
                    nc.tensor.transpose(
                        m_ps, G_sb[a][:, b_ * 128 : (b_ + 1) * 128], ident32
                    )
                    nc.vector.tensor_copy(
                        G_sb[b_][:, a * 128 : (a + 1) * 128], m_ps
                    )
            for a in range(CC):
                nc.vector.tensor_copy(G16[a], G_sb[a])

            # ---- phase 2b: M1_g = G @ Wk_g ; A_g = Wqs_g^T @ M1_g ----
            for g in range(G):
                m1_g = []
                for a in range(CC):
                    m1_ps = ps_v.tile(
                        [128, GC], fp32, tag="vps", name=f"m1ps_{g}_{a}"
                    )
                    for b_ in range(CC):
                        nc.tensor.matmul(
                            m1_ps,
                            G16[b_][:, a * 128 : (a + 1) * 128],
                            wq_sb[b_][:, 768 + g * GC : 768 + (g + 1) * GC],
                            start=(b_ == 0),
                            stop=(b_ == CC - 1),
                        )
                    m1_t = m1_pool.tile(
                        [128, GC], fp16, tag="m1", name=f"m1_{g}_{a}"
                    )
                    nc.vector.tensor_copy(m1_t, m1_ps)
                    m1_g.append(m1_t)

                a_ps = ps_tp.tile([GC, GC], fp32, tag="tp", name=f"aps_{g}")
                for a in range(CC):
                    nc.tensor.matmul(
                        a_ps,
                        wq_sb[a][:, g * GC : (g + 1) * GC],
                        m1_g[a],
                        start=(a == 0),
                        stop=(a == CC - 1),
                    )

                # softmax over free dim + transpose
                nm = sm_pool.tile([GC, 1], fp32, tag="nm", name=f"nm_{g}")
                nc.vector.tensor_reduce(
                    out=nm,
                    in_=a_ps,
                    axis=mybir.AxisListType.X,
                    op=mybir.AluOpType.max,
                    negate=True,
                )
                e_t = sm_pool.tile([GC, GC], fp32, tag="e", name=f"e_{g}")
                ssum = sm_pool.tile([GC, 1], fp32, tag="ssum", name=f"ssum_{g}")
                nc.scalar.activation(
                    e_t,
                    a_ps,
                    mybir.ActivationFunctionType.Exp,
                    bias=nm,
                    scale=1.0,
                    accum_out=ssum,
                )
                rs = sm_pool.tile([GC, 1], fp32, tag="rs", name=f"rs_{g}")
                nc.vector.reciprocal(rs, ssum)
                e16 = sm_pool.tile([GC, GC], fp16, tag="e16", name=f"e16_{g}")
                nc.vector.tensor_scalar_mul(e16, e_t, rs)
                et_ps = ps_tp.tile([GC, GC], fp16, tag="tp", name=f"etps_{g}")
                nc.tensor.transpose(et_ps, e16, ident16[:GC, :GC])
                nc.vector.tensor_copy(eT[g], et_ps)

            # ---- phase 2c: scatter eT blocks into block-diagonal E16 ----
            # eT[g][r, q] = attn_sm[96g+q, 96g+r]
            # E16[(dd,cc)][p, m] = attn_sm[128cc+m, 128dd+p] for matching g
            for g in range(G):
                lo, hi = g * GC, (g + 1) * GC
                for dd in range(lo // 128, (hi - 1) // 128 + 1):
                    r0 = max(0, 128 * dd - lo)
                    r1 = min(GC, 128 * (dd + 1) - lo)
                    for cc_i in range(lo // 128, (hi - 1) // 128 + 1):
                        q0 = max(0, 128 * cc_i - lo)
                        q1 = min(GC, 128 * (cc_i + 1) - lo)
                        nc.sync.dma_start(
                            out=E16[(dd, cc_i)][
                                lo + r0 - 128 * dd : lo + r1 - 128 * dd,
                                lo + q0 - 128 * cc_i : lo + q1 - 128 * cc_i,
                            ],
                            in_=eT[g][r0:r1, q0:q1],
                        )

            # ---- phase 3: out_T = E^T @ vT (per c-chunk) ; proj ; bias ----
            for s in range(NST):
                oT6 = []
                for cc_i in range(CC):
                    dds = sorted(dd for (dd, c2) in CHUNK_PAIRS if c2 == cc_i)
                    o_ps = ps_v.tile(
                        [128, ST], fp32, tag="vps", name=f"ops_{s}_{cc_i}"
                    )
                    for j, dd in enumerate(dds):
                        nc.tensor.matmul(
                            o_ps,
                            E16[(dd, cc_i)],
                            vT6[dd][:, s * ST : (s + 1) * ST],
                            start=(j == 0),
                            stop=(j == len(dds) - 1),
                        )
                    oT_t = oT_pool.tile(
                        [128, ST], fp16, tag="oT", name=f"oT_{s}_{cc_i}"
                    )
                    nc.vector.tensor_copy(oT_t, o_ps)
                    oT6.append(oT_t)
                for t in range(NSUB):
                    y_sb = ysb_pool.tile(
                        [128, C], fp32, tag="ysb", name=f"ysb_{s}_{t}"
                    )
                    for half in range(2):
                        hsl = slice(half * 384, (half + 1) * 384)
                        y_ps = ps_gram.tile(
                            [128, 384], fp32, tag="gram", name=f"yps_{s}_{t}_{half}"
                        )
                        for cc_i in range(CC):
                            nc.tensor.matmul(
                                y_ps,
                                oT6[cc_i][:, t * 128 : (t + 1) * 128],
                                wp_sb[cc_i][:, hsl],
                                start=(cc_i == 0),
                                stop=(cc_i == CC - 1),
                            )
                        nc.vector.tensor_add(y_sb[:, hsl], y_ps, bias_sb[:, hsl])
                    row0 = s * ST + t * 128
                    nc.scalar.dma_start(out=y[row0 : row0 + 128, :], in_=y_sb)

    nc.compile()
    return nc


def _get_nc():
    if "nc" not in _CACHE:
        _CACHE["nc"] = _build_nc()
    return _CACHE["nc"]


def _host_prep(x, w_qkv, w_proj, b_proj):
    x = np.asarray(x, dtype=np.float32)
    w_qkv = np.asarray(w_qkv, dtype=np.float32)
    w_proj = np.asarray(w_proj, dtype=np.float32)
    b_proj = np.asarray(b_proj, dtype=np.float32)

    wq = w_qkv.copy()
    wq[:C, :] *= np.float32(QSCALE)
    wqkvT_h = np.ascontiguousarray(wq.T).astype(np.float16)       # [768, 2304]
    wprojT_h = np.ascontiguousarray(w_proj.T).astype(np.float16)  # [768, 768]

    in_maps = []
    for b_ in range(NCORES):
        in_maps.append(
            {
                "xh": np.ascontiguousarray(x[b_]).astype(np.float16),
                "wqkvT": wqkvT_h,
                "wprojT": wprojT_h,
                "bproj": b_proj,
            }
        )
    return in_maps


def _run(in_maps, trace=False):
    from concourse.bass_utils import run_bass_kernel_spmd

    nc = _get_nc()
    res = run_bass_kernel_spmd(nc, in_maps, list(range(NCORES)), trace=trace)
    out = np.stack([res.results[i]["y"] for i in range(NCORES)], axis=0)
    return out.astype(np.float32, copy=False), res


def kernel(x, w_qkv, w_proj, b_proj):
    in_maps = _host_prep(x, w_qkv, w_proj, b_proj)
    out, _ = _run(in_maps, trace=False)
    return out


def run_profiled(x, w_qkv, w_proj, b_proj):
    """Returns (out, BassKernelResults) with NTFF profiling enabled."""
    in_maps = _host_prep(x, w_qkv, w_proj, b_proj)
    return _run(in_maps, trace=True)


# revision 10
# speedup vs baseline: 1.7789x; 1.3767x over previous
"""ChannelAttention Trainium2 kernel (Bass/Tile), data-parallel over batch.

Problem shapes (hardcoded):
  x      [8, 4096, 768] fp32
  w_qkv  [2304, 768]    fp32
  w_proj [768, 768]     fp32
  b_proj [768]          fp32
  out    [8, 4096, 768] fp32

Reference (per batch b, 8 groups of 96 channels):
  qkv = x @ w_qkv.T ; q *= N**-0.5
  attn_g = softmax(q_g.T @ k_g, axis=-1)     # [96, 96], contracts over N
  out_g  = attn_g @ v_g.T                    # [96, N]
  y = out @ w_proj.T + b_proj

Sharding: batch b -> core b (8 cores SPMD, no collectives).

Algebraic restructure (v4): channel attention collapses around two small
matrices --
  G = X^T X                      [768, 768]   (Gram, symmetric)
  attn_g = softmax(Wq_s G Wk^T)  (per group, [96, 96])
  M = Wv^T BD(attn)^T WprojT     [768, 768]
  y = x @ M + b_proj
so the per-token work is ONE 768-contraction pass for G (using x in
natural layout) and ONE for y (using x^T), plus O(768^3)-ish small
matmuls once per core. All matmul operands fp16 (full PE rate), fp32
accumulation in PSUM; softmax in fp32.
  - G accumulated over token supertiles; only upper 128-chunk blocks
    computed, lower mirrored by PE transpose (G symmetric).
  - x^T tiles produced by PE transpose of the natural x tiles (kept
    resident for the final y = x @ M pass).
  - P = BD(attn)^T WprojT per group (lhsT = softmax block directly),
    rows assembled into 128-aligned d-chunks via partition-shifting
    SBUF->SBUF DMA copies; M = Wv^T P with Wv in natural [d, a] layout.
Host pre-work: fp16 casts, fold N**-0.5 into Wq, transpose of the q/k
weight halves and of w_proj (layout prep only).
"""

import numpy as np

B, N, C = 8, 4096, 768
G = 8
GC = C // G          # 96
NCORES = 8
ST = 512             # tokens per supertile
NST = N // ST        # 8
NSUB = ST // 128     # 4
CC = C // 128        # 6 chunks of the channel dim
QSCALE = float(N) ** -0.5  # 1/64

_CACHE = {}


def _build_nc():
    import concourse.bass as bass
    import concourse.mybir as mybir
    import concourse.tile as tile
    from concourse import bacc
    from concourse.masks import make_identity

    fp16 = mybir.dt.float16
    fp32 = mybir.dt.float32

    nc = bacc.Bacc(
        "TRN2", target_bir_lowering=False, debug=False, num_devices=NCORES
    )

    xh = nc.dram_tensor("xh", [N, C], fp16, kind="ExternalInput").ap()
    # q/k halves of w_qkv, transposed to [c, 2*768], q pre-scaled
    wqkT = nc.dram_tensor("wqkT", [C, 2 * C], fp16, kind="ExternalInput").ap()
    # v rows of w_qkv in natural [d, a] layout
    wv = nc.dram_tensor("wv", [C, C], fp16, kind="ExternalInput").ap()
    wprojT = nc.dram_tensor("wprojT", [C, C], fp16, kind="ExternalInput").ap()
    bproj = nc.dram_tensor("bproj", [C], fp32, kind="ExternalInput").ap()
    y = nc.dram_tensor("y", [N, C], fp32, kind="ExternalOutput").ap()

    with tile.TileContext(nc) as tc:
        from contextlib import ExitStack

        with ExitStack() as ctx:
            weights = ctx.enter_context(tc.tile_pool(name="weights", bufs=1))
            persist = ctx.enter_context(tc.tile_pool(name="persist", bufs=1))
            xn_pool = ctx.enter_context(tc.tile_pool(name="xn", bufs=10))
            ysb_pool = ctx.enter_context(tc.tile_pool(name="ysb", bufs=4))
            sm_pool = ctx.enter_context(tc.tile_pool(name="sm", bufs=4))
            m1_pool = ctx.enter_context(tc.tile_pool(name="m1", bufs=1))
            ps_gram = ctx.enter_context(
                tc.tile_pool(name="ps_gram", bufs=3, space="PSUM")
            )
            ps_big = ctx.enter_context(
                tc.tile_pool(name="ps_big", bufs=3, space="PSUM")
            )
            ps_tp = ctx.enter_context(
                tc.tile_pool(name="ps_tp", bufs=2, space="PSUM")
            )

            # ---- static weights ----
            wqk_sb = []
            for a in range(CC):
                wtile = weights.tile([128, 2 * C], fp16, name=f"wqk_{a}")
                nc.sync.dma_start(
                    out=wtile, in_=wqkT[a * 128 : (a + 1) * 128, :]
                )
                wqk_sb.append(wtile)
            wv_sb = []
            for dd in range(CC):
                wvtile = weights.tile([128, C], fp16, name=f"wv_{dd}")
                nc.sync.dma_start(out=wvtile, in_=wv[dd * 128 : (dd + 1) * 128, :])
                wv_sb.append(wvtile)
            wpg_sb = []
            for g in range(G):
                wpg = weights.tile([GC, C], fp16, name=f"wpg_{g}")
                nc.sync.dma_start(out=wpg, in_=wprojT[g * GC : (g + 1) * GC, :])
                wpg_sb.append(wpg)
            bias_sb = weights.tile([128, C], fp32, name="bias_sb")
            bias_bcast = bass.AP(
                tensor=bproj.tensor,
                offset=bproj.offset,
                ap=[[0, 128]] + [list(p) for p in bproj.ap],
            )
            nc.gpsimd.dma_start(out=bias_sb, in_=bias_bcast)
            ident16 = weights.tile([128, 128], fp16, name="ident16")
            make_identity(nc, ident16)
            ident32 = weights.tile([128, 128], fp32, name="ident32")
            make_identity(nc, ident32)

            # ---- persistent intermediates ----
            G_sb = [
                persist.tile([128, C], fp32, name=f"G_{a}") for a in range(CC)
            ]
            G16 = [
                persist.tile([128, C], fp16, name=f"G16_{a}") for a in range(CC)
            ]
            xT6 = [
                persist.tile([128, N], fp16, name=f"xT_{a}") for a in range(CC)
            ]
            e16 = [
                persist.tile([GC, GC], fp16, name=f"e16_{g}") for g in range(G)
            ]
            P6 = [persist.tile([128, C], fp16, name=f"P_{dd}") for dd in range(CC)]
            M_sb = [
                persist.tile([128, C], fp16, name=f"M_{a}") for a in range(CC)
            ]

            def gram_slices(a):
                out = []
                off = a * 128
                while off < C:
                    w = min(384, C - off)
                    out.append((off, w))
                    off += w
                return out

            # ---- phase 1: Gram accumulation + x^T materialization ----
            for s in range(NST):
                xn = []
                for t in range(NSUB):
                    xtile = xn_pool.tile(
                        [128, C], fp16, tag="xn", name=f"xn_{s}_{t}"
                    )
                    r0 = s * ST + t * 128
                    nc.scalar.dma_start(out=xtile, in_=xh[r0 : r0 + 128, :])
                    xn.append(xtile)

                for t in range(NSUB):
                    for a in range(CC):
                        tp_ps = ps_tp.tile(
                            [128, 128], fp16, tag="tp", name=f"tp_{s}_{t}_{a}"
                        )
                        nc.tensor.transpose(
                            tp_ps, xn[t][:, a * 128 : (a + 1) * 128], ident16
                        )
                        r0 = s * ST + t * 128
                        nc.vector.tensor_copy(xT6[a][:, r0 : r0 + 128], tp_ps)

                for a in range(CC):
                    for (off, w) in gram_slices(a):
                        g_ps = ps_gram.tile(
                            [128, 384], fp32, tag="gram", name=f"gps_{s}_{a}_{off}"
                        )
                        for t in range(NSUB):
                            nc.tensor.matmul(
                                g_ps[:, :w],
                                xn[t][:, a * 128 : (a + 1) * 128],
                                xn[t][:, off : off + w],
                                start=(t == 0),
                                stop=(t == NSUB - 1),
                            )
                        if s == 0:
                            nc.vector.tensor_copy(
                                G_sb[a][:, off : off + w], g_ps[:, :w]
                            )
                        else:
                            nc.vector.tensor_add(
                                G_sb[a][:, off : off + w],
                                G_sb[a][:, off : off + w],
                                g_ps[:, :w],
                            )

            # ---- phase 2a: mirror lower G blocks, cast to fp16 ----
            for a in range(CC):
                for b_ in range(a + 1, CC):
                    m_ps = ps_gram.tile(
                        [128, 128], fp32, tag="gram", name=f"mir_{a}_{b_}"
                    )
                    nc.tensor.transpose(
                        m_ps, G_sb[a][:, b_ * 128 : (b_ + 1) * 128], ident32
                    )
                    nc.vector.tensor_copy(
                        G_sb[b_][:, a * 128 : (a + 1) * 128], m_ps
                    )
            for a in range(CC):
                nc.vector.tensor_copy(G16[a], G_sb[a])

            # ---- phase 2b: logits A_g = Wq_s^T (G Wk_g^T), softmax ----
            for g in range(G):
                m1_g = []
                for a in range(CC):
                    m1_ps = ps_gram.tile(
                        [128, GC], fp32, tag="gram", name=f"m1ps_{g}_{a}"
                    )
                    for b_ in range(CC):
                        nc.tensor.matmul(
                            m1_ps,
                            G16[b_][:, a * 128 : (a + 1) * 128],
                            wqk_sb[b_][:, 768 + g * GC : 768 + (g + 1) * GC],
                            start=(b_ == 0),
                            stop=(b_ == CC - 1),
                        )
                    m1_t = m1_pool.tile(
                        [128, GC], fp16, tag="m1", name=f"m1_{g}_{a}"
                    )
                    nc.vector.tensor_copy(m1_t, m1_ps)
                    m1_g.append(m1_t)

                a_ps = ps_tp.tile([GC, GC], fp32, tag="tp", name=f"aps_{g}")
                for a in range(CC):
                    nc.tensor.matmul(
                        a_ps,
                        wqk_sb[a][:, g * GC : (g + 1) * GC],
                        m1_g[a],
                        start=(a == 0),
                        stop=(a == CC - 1),
                    )

                nm = sm_pool.tile([GC, 1], fp32, tag="nm", name=f"nm_{g}")
                nc.vector.tensor_reduce(
                    out=nm,
                    in_=a_ps,
                    axis=mybir.AxisListType.X,
                    op=mybir.AluOpType.max,
                    negate=True,
                )
                e_t = sm_pool.tile([GC, GC], fp32, tag="e", name=f"e_{g}")
                ssum = sm_pool.tile([GC, 1], fp32, tag="ssum", name=f"ssum_{g}")
                nc.scalar.activation(
                    e_t,
                    a_ps,
                    mybir.ActivationFunctionType.Exp,
                    bias=nm,
                    scale=1.0,
                    accum_out=ssum,
                )
                rs = sm_pool.tile([GC, 1], fp32, tag="rs", name=f"rs_{g}")
                nc.vector.reciprocal(rs, ssum)
                nc.vector.tensor_scalar_mul(e16[g], e_t, rs)

            # ---- phase 2c: P = BD(attn)^T WprojT ; M = Wv^T P ----
            # P_g[d_local, e] = sum_c e16_g[c, d_local] * wprojT[96g+c, e]
            for g in range(G):
                for half in range(2):
                    hsl = slice(half * 384, (half + 1) * 384)
                    p_ps = ps_big.tile(
                        [GC, 384], fp32, tag="big", name=f"pps_{g}_{half}"
                    )
                    nc.tensor.matmul(
                        p_ps, e16[g], wpg_sb[g][:, hsl], start=True, stop=True
                    )
                    pg16 = sm_pool.tile(
                        [GC, 384], fp16, tag="pg", name=f"pg_{g}_{half}"
                    )
                    nc.vector.tensor_copy(pg16, p_ps)
                    # scatter rows into 128-aligned P6 chunks
                    lo = g * GC
                    for dd in range(lo // 128, (lo + GC - 1) // 128 + 1):
                        r0 = max(0, 128 * dd - lo)
                        r1 = min(GC, 128 * (dd + 1) - lo)
                        nc.sync.dma_start(
                            out=P6[dd][lo + r0 - 128 * dd : lo + r1 - 128 * dd, hsl],
                            in_=pg16[r0:r1, :],
                        )

            for ab in range(CC):
                for half in range(2):
                    hsl = slice(half * 384, (half + 1) * 384)
                    m_ps = ps_big.tile(
                        [128, 384], fp32, tag="big", name=f"mps_{ab}_{half}"
                    )
                    for dd in range(CC):
                        nc.tensor.matmul(
                            m_ps,
                            wv_sb[dd][:, ab * 128 : (ab + 1) * 128],
                            P6[dd][:, hsl],
                            start=(dd == 0),
                            stop=(dd == CC - 1),
                        )
                    nc.vector.tensor_copy(M_sb[ab][:, hsl], m_ps)

            # ---- phase 3: y = x @ M + b ----
            for s in range(NST):
                for t in range(NSUB):
                    r0 = s * ST + t * 128
                    y_sb = ysb_pool.tile(
                        [128, C], fp32, tag="ysb", name=f"ysb_{s}_{t}"
                    )
                    for half in range(2):
                        hsl = slice(half * 384, (half + 1) * 384)
                        y_ps = ps_big.tile(
                            [128, 384], fp32, tag="big", name=f"yps_{s}_{t}_{half}"
                        )
                        for a in range(CC):
                            nc.tensor.matmul(
                                y_ps,
                                xT6[a][:, r0 : r0 + 128],
                                M_sb[a][:, hsl],
                                start=(a == 0),
                                stop=(a == CC - 1),
                            )
                        nc.vector.tensor_add(y_sb[:, hsl], y_ps, bias_sb[:, hsl])
                    nc.scalar.dma_start(out=y[r0 : r0 + 128, :], in_=y_sb)

    nc.compile()
    return nc


def _get_nc():
    if "nc" not in _CACHE:
        _CACHE["nc"] = _build_nc()
    return _CACHE["nc"]


def _host_prep(x, w_qkv, w_proj, b_proj):
    x = np.asarray(x, dtype=np.float32)
    w_qkv = np.asarray(w_qkv, dtype=np.float32)
    w_proj = np.asarray(w_proj, dtype=np.float32)
    b_proj = np.asarray(b_proj, dtype=np.float32)

    wqk = w_qkv[: 2 * C, :].copy()
    wqk[:C, :] *= np.float32(QSCALE)
    wqkT_h = np.ascontiguousarray(wqk.T).astype(np.float16)       # [768, 1536]
    wv_h = np.ascontiguousarray(w_qkv[2 * C :, :]).astype(np.float16)
    wprojT_h = np.ascontiguousarray(w_proj.T).astype(np.float16)  # [768, 768]

    in_maps = []
    for b_ in range(NCORES):
        in_maps.append(
            {
                "xh": np.ascontiguousarray(x[b_]).astype(np.float16),
                "wqkT": wqkT_h,
                "wv": wv_h,
                "wprojT": wprojT_h,
                "bproj": b_proj,
            }
        )
    return in_maps


def _run(in_maps, trace=False):
    from concourse.bass_utils import run_bass_kernel_spmd

    nc = _get_nc()
    res = run_bass_kernel_spmd(nc, in_maps, list(range(NCORES)), trace=trace)
    out = np.stack([res.results[i]["y"] for i in range(NCORES)], axis=0)
    return out.astype(np.float32, copy=False), res


def kernel(x, w_qkv, w_proj, b_proj):
    in_maps = _host_prep(x, w_qkv, w_proj, b_proj)
    out, _ = _run(in_maps, trace=False)
    return out


def run_profiled(x, w_qkv, w_proj, b_proj):
    """Returns (out, BassKernelResults) with NTFF profiling enabled."""
    in_maps = _host_prep(x, w_qkv, w_proj, b_proj)
    return _run(in_maps, trace=True)


# revision 12
# speedup vs baseline: 1.9548x; 1.0989x over previous
"""ChannelAttention Trainium2 kernel (Bass/Tile), data-parallel over batch.

Problem shapes (hardcoded):
  x      [8, 4096, 768] fp32
  w_qkv  [2304, 768]    fp32
  w_proj [768, 768]     fp32
  b_proj [768]          fp32
  out    [8, 4096, 768] fp32

Reference (per batch b, 8 groups of 96 channels):
  qkv = x @ w_qkv.T ; q *= N**-0.5
  attn_g = softmax(q_g.T @ k_g, axis=-1)     # [96, 96], contracts over N
  out_g  = attn_g @ v_g.T                    # [96, N]
  y = out @ w_proj.T + b_proj

Sharding: batch b -> core b (8 cores SPMD, no collectives).

Algebraic restructure (v4): channel attention collapses around two small
matrices --
  G = X^T X                      [768, 768]   (Gram, symmetric)
  attn_g = softmax(Wq_s G Wk^T)  (per group, [96, 96])
  M = Wv^T BD(attn)^T WprojT     [768, 768]
  y = x @ M + b_proj
so the per-token work is ONE 768-contraction pass for G (using x in
natural layout) and ONE for y (using x^T), plus O(768^3)-ish small
matmuls once per core. All matmul operands fp16 (full PE rate), fp32
accumulation in PSUM; softmax in fp32.
  - G accumulated over token supertiles; only upper 128-chunk blocks
    computed, lower mirrored by PE transpose (G symmetric).
  - x^T tiles produced by PE transpose of the natural x tiles (kept
    resident for the final y = x @ M pass).
  - P = BD(attn)^T WprojT per group (lhsT = softmax block directly),
    rows assembled into 128-aligned d-chunks via partition-shifting
    SBUF->SBUF DMA copies; M = Wv^T P with Wv in natural [d, a] layout.
Host pre-work: fp16 casts, fold N**-0.5 into Wq, transpose of the q/k
weight halves and of w_proj (layout prep only).
"""

import numpy as np

B, N, C = 8, 4096, 768
G = 8
GC = C // G          # 96
NCORES = 8
ST = 512             # tokens per supertile
NST = N // ST        # 8
NSUB = ST // 128     # 4
CC = C // 128        # 6 chunks of the channel dim
QSCALE = float(N) ** -0.5  # 1/64

_CACHE = {}


def _build_nc():
    import concourse.bass as bass
    import concourse.mybir as mybir
    import concourse.tile as tile
    from concourse import bacc
    from concourse.masks import make_identity

    fp16 = mybir.dt.float16
    fp32 = mybir.dt.float32

    nc = bacc.Bacc(
        "TRN2", target_bir_lowering=False, debug=False, num_devices=NCORES
    )

    xh = nc.dram_tensor("xh", [N, C], fp16, kind="ExternalInput").ap()
    # q/k halves of w_qkv, transposed to [c, 2*768], q pre-scaled
    wqkT = nc.dram_tensor("wqkT", [C, 2 * C], fp16, kind="ExternalInput").ap()
    # v rows of w_qkv in natural [d, a] layout
    wv = nc.dram_tensor("wv", [C, C], fp16, kind="ExternalInput").ap()
    wprojT = nc.dram_tensor("wprojT", [C, C], fp16, kind="ExternalInput").ap()
    bproj = nc.dram_tensor("bproj", [C], fp32, kind="ExternalInput").ap()
    y = nc.dram_tensor("y", [N, C], fp32, kind="ExternalOutput").ap()

    with tile.TileContext(nc) as tc:
        from contextlib import ExitStack

        with ExitStack() as ctx:
            weights = ctx.enter_context(tc.tile_pool(name="weights", bufs=1))
            persist = ctx.enter_context(tc.tile_pool(name="persist", bufs=1))
            xn_pool = ctx.enter_context(tc.tile_pool(name="xn", bufs=10))
            ysb_pool = ctx.enter_context(tc.tile_pool(name="ysb", bufs=4))
            sm_pool = ctx.enter_context(tc.tile_pool(name="sm", bufs=4))
            ps_gram = ctx.enter_context(
                tc.tile_pool(name="ps_gram", bufs=3, space="PSUM")
            )
            ps_big = ctx.enter_context(
                tc.tile_pool(name="ps_big", bufs=5, space="PSUM")
            )

            # ---- static weight tiles (DMAs issued after phase 1 so the
            # token stream owns the HBM early) ----
            wqk_sb = [
                weights.tile([128, 2 * C], fp16, name=f"wqk_{a}")
                for a in range(CC)
            ]
            wv_sb = [
                weights.tile([128, C], fp16, name=f"wv_{dd}") for dd in range(CC)
            ]
            wpg_sb = [
                weights.tile([GC, C], fp16, name=f"wpg_{g}") for g in range(G)
            ]
            bias_sb = weights.tile([128, C], fp32, name="bias_sb")
            bias_bcast = bass.AP(
                tensor=bproj.tensor,
                offset=bproj.offset,
                ap=[[0, 128]] + [list(p) for p in bproj.ap],
            )
            nc.gpsimd.dma_start(out=bias_sb, in_=bias_bcast)
            ident16 = weights.tile([128, 128], fp16, name="ident16")
            make_identity(nc, ident16)
            ident32 = weights.tile([128, 128], fp32, name="ident32")
            make_identity(nc, ident32)

            # ---- persistent intermediates ----
            G_sb = [
                persist.tile([128, C], fp32, name=f"G_{a}") for a in range(CC)
            ]
            G16 = [
                persist.tile([128, C], fp16, name=f"G16_{a}") for a in range(CC)
            ]
            xT6 = [
                persist.tile([128, N], fp16, name=f"xT_{a}") for a in range(CC)
            ]
            e16 = [
                persist.tile([GC, GC], fp16, name=f"e16_{g}") for g in range(G)
            ]
            P6 = [persist.tile([128, C], fp16, name=f"P_{dd}") for dd in range(CC)]
            M_sb = [
                persist.tile([128, C], fp16, name=f"M_{a}") for a in range(CC)
            ]

            def gram_slices(a):
                out = []
                off = a * 128
                while off < C:
                    w = min(384, C - off)
                    out.append((off, w))
                    off += w
                return out

            # ---- phase 1: Gram accumulation + x^T materialization ----
            for s in range(NST):
                xn = []
                for t in range(NSUB):
                    xtile = xn_pool.tile(
                        [128, C], fp16, tag="xn", name=f"xn_{s}_{t}"
                    )
                    r0 = s * ST + t * 128
                    nc.scalar.dma_start(out=xtile, in_=xh[r0 : r0 + 128, :])
                    xn.append(xtile)

                for t in range(NSUB):
                    for a in range(CC):
                        tp_ps = ps_big.tile(
                            [128, 128], fp16, tag="big", name=f"tp_{s}_{t}_{a}"
                        )
                        nc.tensor.transpose(
                            tp_ps, xn[t][:, a * 128 : (a + 1) * 128], ident16
                        )
                        r0 = s * ST + t * 128
                        nc.vector.tensor_copy(xT6[a][:, r0 : r0 + 128], tp_ps)

                for a in range(CC):
                    for (off, w) in gram_slices(a):
                        g_ps = ps_gram.tile(
                            [128, 384], fp32, tag="gram", name=f"gps_{s}_{a}_{off}"
                        )
                        for t in range(NSUB):
                            nc.tensor.matmul(
                                g_ps[:, :w],
                                xn[t][:, a * 128 : (a + 1) * 128],
                                xn[t][:, off : off + w],
                                start=(t == 0),
                                stop=(t == NSUB - 1),
                            )
                        if s == 0:
                            nc.vector.tensor_copy(
                                G_sb[a][:, off : off + w], g_ps[:, :w]
                            )
                        else:
                            nc.vector.tensor_add(
                                G_sb[a][:, off : off + w],
                                G_sb[a][:, off : off + w],
                                g_ps[:, :w],
                            )

            # weight loads (needed from phase 2 on)
            for a in range(CC):
                nc.sync.dma_start(
                    out=wqk_sb[a], in_=wqkT[a * 128 : (a + 1) * 128, :]
                )
            for dd in range(CC):
                nc.sync.dma_start(
                    out=wv_sb[dd], in_=wv[dd * 128 : (dd + 1) * 128, :]
                )
            for g in range(G):
                nc.sync.dma_start(
                    out=wpg_sb[g], in_=wprojT[g * GC : (g + 1) * GC, :]
                )

            # ---- phase 2a: mirror lower G blocks, cast to fp16 ----
            for a in range(CC):
                for b_ in range(a + 1, CC):
                    m_ps = ps_gram.tile(
                        [128, 128], fp32, tag="gram", name=f"mir_{a}_{b_}"
                    )
                    nc.tensor.transpose(
                        m_ps, G_sb[a][:, b_ * 128 : (b_ + 1) * 128], ident32
                    )
                    nc.vector.tensor_copy(
                        G_sb[b_][:, a * 128 : (a + 1) * 128], m_ps
                    )
            for a in range(CC):
                nc.vector.tensor_copy(G16[a], G_sb[a])

            # ---- phase 2b: M1 = G Wk^T (all groups batched), then per
            # group A_g = Wq_s_g^T M1_g, softmax ----
            M1_sb = [
                persist.tile([128, C], fp16, name=f"m1_{a}") for a in range(CC)
            ]
            for a in range(CC):
                for half in range(2):
                    hsl = slice(half * 384, (half + 1) * 384)
                    m1_ps = ps_gram.tile(
                        [128, 384], fp32, tag="gram", name=f"m1ps_{a}_{half}"
                    )
                    for b_ in range(CC):
                        nc.tensor.matmul(
                            m1_ps,
                            G16[b_][:, a * 128 : (a + 1) * 128],
                            wqk_sb[b_][:, 768 + half * 384 : 768 + (half + 1) * 384],
                            start=(b_ == 0),
                            stop=(b_ == CC - 1),
                        )
                    nc.vector.tensor_copy(M1_sb[a][:, hsl], m1_ps)

            for g in range(G):
                a_ps = ps_big.tile([GC, GC], fp32, tag="big", name=f"aps_{g}")
                for a in range(CC):
                    nc.tensor.matmul(
                        a_ps,
                        wqk_sb[a][:, g * GC : (g + 1) * GC],
                        M1_sb[a][:, g * GC : (g + 1) * GC],
                        start=(a == 0),
                        stop=(a == CC - 1),
                    )

                nm = sm_pool.tile([GC, 1], fp32, tag="nm", name=f"nm_{g}")
                nc.vector.tensor_reduce(
                    out=nm,
                    in_=a_ps,
                    axis=mybir.AxisListType.X,
                    op=mybir.AluOpType.max,
                    negate=True,
                )
                e_t = sm_pool.tile([GC, GC], fp32, tag="e", name=f"e_{g}")
                ssum = sm_pool.tile([GC, 1], fp32, tag="ssum", name=f"ssum_{g}")
                nc.scalar.activation(
                    e_t,
                    a_ps,
                    mybir.ActivationFunctionType.Exp,
                    bias=nm,
                    scale=1.0,
                    accum_out=ssum,
                )
                rs = sm_pool.tile([GC, 1], fp32, tag="rs", name=f"rs_{g}")
                nc.vector.reciprocal(rs, ssum)
                nc.vector.tensor_scalar_mul(e16[g], e_t, rs)

            # ---- phase 2c: P = BD(attn)^T WprojT ; M = Wv^T P ----
            # P_g[d_local, e] = sum_c e16_g[c, d_local] * wprojT[96g+c, e]
            for g in range(G):
                for half in range(2):
                    hsl = slice(half * 384, (half + 1) * 384)
                    p_ps = ps_big.tile(
                        [GC, 384], fp32, tag="big", name=f"pps_{g}_{half}"
                    )
                    nc.tensor.matmul(
                        p_ps, e16[g], wpg_sb[g][:, hsl], start=True, stop=True
                    )
                    pg16 = sm_pool.tile(
                        [GC, 384], fp16, tag="pg", name=f"pg_{g}_{half}"
                    )
                    nc.vector.tensor_copy(pg16, p_ps)
                    # scatter rows into 128-aligned P6 chunks
                    lo = g * GC
                    for dd in range(lo // 128, (lo + GC - 1) // 128 + 1):
                        r0 = max(0, 128 * dd - lo)
                        r1 = min(GC, 128 * (dd + 1) - lo)
                        nc.sync.dma_start(
                            out=P6[dd][lo + r0 - 128 * dd : lo + r1 - 128 * dd, hsl],
                            in_=pg16[r0:r1, :],
                        )

            for ab in range(CC):
                for half in range(2):
                    hsl = slice(half * 384, (half + 1) * 384)
                    m_ps = ps_big.tile(
                        [128, 384], fp32, tag="big", name=f"mps_{ab}_{half}"
                    )
                    for dd in range(CC):
                        nc.tensor.matmul(
                            m_ps,
                            wv_sb[dd][:, ab * 128 : (ab + 1) * 128],
                            P6[dd][:, hsl],
                            start=(dd == 0),
                            stop=(dd == CC - 1),
                        )
                    nc.vector.tensor_copy(M_sb[ab][:, hsl], m_ps)

            # ---- phase 3: y = x @ M + b ----
            for s in range(NST):
                for t in range(NSUB):
                    r0 = s * ST + t * 128
                    y_sb = ysb_pool.tile(
                        [128, C], fp32, tag="ysb", name=f"ysb_{s}_{t}"
                    )
                    for half in range(2):
                        hsl = slice(half * 384, (half + 1) * 384)
                        y_ps = ps_big.tile(
                            [128, 384], fp32, tag="big", name=f"yps_{s}_{t}_{half}"
                        )
                        for a in range(CC):
                            nc.tensor.matmul(
                                y_ps,
                                xT6[a][:, r0 : r0 + 128],
                                M_sb[a][:, hsl],
                                start=(a == 0),
                                stop=(a == CC - 1),
                            )
                        nc.vector.tensor_add(y_sb[:, hsl], y_ps, bias_sb[:, hsl])
                    nc.scalar.dma_start(out=y[r0 : r0 + 128, :], in_=y_sb)

    nc.compile()
    return nc


def _get_nc():
    if "nc" not in _CACHE:
        _CACHE["nc"] = _build_nc()
    return _CACHE["nc"]


def _host_prep(x, w_qkv, w_proj, b_proj):
    x = np.asarray(x, dtype=np.float32)
    w_qkv = np.asarray(w_qkv, dtype=np.float32)
    w_proj = np.asarray(w_proj, dtype=np.float32)
    b_proj = np.asarray(b_proj, dtype=np.float32)

    wqk = w_qkv[: 2 * C, :].copy()
    wqk[:C, :] *= np.float32(QSCALE)
    wqkT_h = np.ascontiguousarray(wqk.T).astype(np.float16)       # [768, 1536]
    wv_h = np.ascontiguousarray(w_qkv[2 * C :, :]).astype(np.float16)
    wprojT_h = np.ascontiguousarray(w_proj.T).astype(np.float16)  # [768, 768]

    in_maps = []
    for b_ in range(NCORES):
        in_maps.append(
            {
                "xh": np.ascontiguousarray(x[b_]).astype(np.float16),
                "wqkT": wqkT_h,
                "wv": wv_h,
                "wprojT": wprojT_h,
                "bproj": b_proj,
            }
        )
    return in_maps


def _run(in_maps, trace=False):
    from concourse.bass_utils import run_bass_kernel_spmd

    nc = _get_nc()
    res = run_bass_kernel_spmd(nc, in_maps, list(range(NCORES)), trace=trace)
    out = np.stack([res.results[i]["y"] for i in range(NCORES)], axis=0)
    return out.astype(np.float32, copy=False), res


def kernel(x, w_qkv, w_proj, b_proj):
    in_maps = _host_prep(x, w_qkv, w_proj, b_proj)
    out, _ = _run(in_maps, trace=False)
    return out


def run_profiled(x, w_qkv, w_proj, b_proj):
    """Returns (out, BassKernelResults) with NTFF profiling enabled."""
    in_maps = _host_prep(x, w_qkv, w_proj, b_proj)
    return _run(in_maps, trace=True)
